# revision 14
# baseline (speedup 1.0000x reference)
"""Trainium2 Bass kernel for nn_Head (NTM-style addressing head).

Data-parallel over batch: 8 cores x 128 samples. Each core computes its
[128, 16384] slice of the output with zero collectives.

Self-contained: only imports numpy + the installed concourse stack.
"""

import sys

if "/opt/trn_rl_repo" not in sys.path:
    sys.path.insert(0, "/opt/trn_rl_repo")

from contextlib import ExitStack

import numpy as np

import concourse.bass as bass
import concourse.tile as tile
from concourse import bacc, mybir
from concourse.bass_utils import run_bass_kernel_spmd
from concourse.masks import make_identity
from concourse.tile import add_dep_helper

B, H, N, M = 1024, 512, 16384, 64
NCORES = 8
BL = B // NCORES  # 128 samples per core == partition count
HALF = N // 2  # 8192
NCH = 8  # elementwise chunks over N
CH = N // NCH  # 2048
F32 = mybir.dt.float32
BF16 = mybir.dt.bfloat16
AF = mybir.ActivationFunctionType
OP = mybir.AluOpType
AX = mybir.AxisListType
GMAX = 1.0 - 2.0**-23  # clamp for g so (1-g) stays representable
import os
USE_ARS = os.environ.get("USE_ARS", "1") == "1"


def _body(ctx: ExitStack, tc: tile.TileContext, out, h, wprev, mtp, wcat, bcat):
    nc = tc.nc

    const = ctx.enter_context(tc.tile_pool(name="const", bufs=1))
    small = ctx.enter_context(tc.tile_pool(name="small", bufs=1))
    big = ctx.enter_context(tc.tile_pool(name="big", bufs=1))
    sq_pool = ctx.enter_context(tc.tile_pool(name="sq_pool", bufs=2))
    psum = ctx.enter_context(tc.tile_pool(name="psum", bufs=2, space="PSUM"))

    # ---- m first: its DMA + norm pipeline is the phase-A long pole ----
    mp = big.tile([128, HALF], F32, name="mp", tag="mp_vb")
    nc.sync.dma_start(out=mp, in_=mtp)

    # ---- constants ----
    ident = const.tile([128, 128], F32, name="ident")
    make_identity(nc, ident)
    bias_sb = const.tile([128, 70], F32, name="bias_sb")
    nc.gpsimd.dma_start(out=bias_sb, in_=bcat.to_broadcast((128, 70)))
    wcat_sb = const.tile([128, 4, 70], F32, name="wcat_sb")
    nc.sync.dma_start(out=wcat_sb, in_=wcat.rearrange("(j p) c -> p j c", p=128))
    ones2 = const.tile([128, 64], BF16, name="ones2")
    nc.vector.memset(ones2, 1.0)

    # ---- h -> hT (4 PE transposes packed into one PSUM tile) ----
    h_sb = small.tile([128, H], F32, name="h_sb")
    nc.sync.dma_start(out=h_sb, in_=h)
    ps_h = psum.tile([128, CH], F32, name="ps_h", tag="ps")
    for j in range(4):
        nc.tensor.transpose(
            ps_h[:, 128 * j : 128 * (j + 1)], h_sb[:, 128 * j : 128 * (j + 1)], ident
        )
    hT = const.tile([128, 512], F32, name="hT")
    nc.vector.tensor_copy(hT, ps_h[:, 0:512])

    # ---- fused head projections: proj = h @ Wcat + bcat  [128, 70] ----
    ps_p = psum.tile([128, CH], F32, name="ps_p", tag="ps")
    for j in range(4):
        nc.tensor.matmul(
            ps_p[:, 0:70],
            lhsT=hT[:, 128 * j : 128 * (j + 1)],
            rhs=wcat_sb[:, j, :],
            start=(j == 0),
            stop=(j == 3),
        )
    proj = small.tile([128, 70], F32, name="proj")
    nc.vector.tensor_tensor(out=proj, in0=ps_p[:, 0:70], in1=bias_sb, op=OP.add)

    # ---- head activations ----
    k_sb = small.tile([128, 64], F32, name="k_sb")
    nc.vector.tensor_scalar(
        out=k_sb, in0=proj[:, 0:64], scalar1=0.0, scalar2=1.0, op0=OP.max, op1=OP.min
    )
    beta = small.tile([128, 1], F32, name="beta")
    nc.vector.tensor_scalar(
        out=beta, in0=proj[:, 64:65], scalar1=0.0, scalar2=None, op0=OP.max
    )
    g_sb = small.tile([128, 1], F32, name="g_sb")
    nc.vector.tensor_scalar(
        out=g_sb, in0=proj[:, 65:66], scalar1=0.0, scalar2=1.0, op0=OP.max, op1=OP.min
    )
    gamma = small.tile([128, 1], F32, name="gamma")
    nc.vector.tensor_scalar(
        out=gamma, in0=proj[:, 69:70], scalar1=0.0, scalar2=1.0, op0=OP.max, op1=OP.add
    )
    # shift softmax over 3 logits (small range -> no max subtraction)
    es = small.tile([128, 3], F32, name="es")
    ssum = small.tile([128, 1], F32, name="ssum")
    nc.scalar.activation(out=es, in_=proj[:, 66:69], func=AF.Exp, accum_out=ssum)
    iss = small.tile([128, 1], F32, name="iss")
    nc.vector.reciprocal(out=iss, in_=ssum)
    s3 = small.tile([128, 3], F32, name="s3")
    nc.vector.tensor_scalar(out=s3, in0=es, scalar1=iss, scalar2=None, op0=OP.mult)

    # ---- bscale = beta / ||k||  (squares on DVE; rsqrt via exp(-0.5 ln)) ----
    scratch64 = small.tile([128, 64], F32, name="scratch64")
    nc.vector.tensor_tensor(out=scratch64, in0=k_sb, in1=k_sb, op=OP.mult)
    ksq = small.tile([128, 1], F32, name="ksq")
    nc.vector.tensor_reduce(out=ksq, in_=scratch64, axis=AX.X, op=OP.add)
    kln = small.tile([128, 1], F32, name="kln")
    nc.scalar.activation(out=kln, in_=ksq, func=AF.Ln)
    invk = small.tile([128, 1], F32, name="invk")
    nc.scalar.activation(out=invk, in_=kln, func=AF.Exp, scale=-0.5)
    bscale = small.tile([128, 1], F32, name="bscale")
    nc.vector.tensor_tensor(out=bscale, in0=beta, in1=invk, op=OP.mult)

    # ---- kT (bf16), duplicated on both partition halves: transpose [k | k] ----
    k2 = small.tile([128, 128], F32, name="k2")
    nc.vector.tensor_copy(k2[:, 0:64], k_sb)
    nc.vector.tensor_copy(k2[:, 64:128], k_sb)
    ps_k = psum.tile([128, CH], F32, name="ps_k", tag="ps")
    nc.tensor.transpose(ps_k[:, 0:128], k2, ident)
    kT = const.tile([128, 128], BF16, name="kT")
    nc.vector.tensor_copy(kT, ps_k[:, 0:128])

    # ---- m: host provides mtp [128, 8192] f32:
    #      rows 0:64  = m^T for n in [0, 8192)
    #      rows 64:128= m^T for n in [8192, 16384)
    #   normalize columns by per-n 1/||m_n|| -> mT_s bf16 ----
    mT = big.tile([128, HALF], BF16, name="mT")
    m_exps = []
    for grp in range(2):  # 2 chunks per group: exactly the 2 PSUM slots
        cs = [2 * grp, 2 * grp + 1]
        inv_pss = []
        for c in cs:
            sl = slice(c * 2048, (c + 1) * 2048)
            psq = sq_pool.tile([128, 2048], BF16, name="psq", tag="psq")
            nc.vector.tensor_tensor(out=psq, in0=mp[:, sl], in1=mp[:, sl], op=OP.mult)
            inv_ps = psum.tile([128, CH], F32, name="inv_ps", tag="ps")
            for s in range(4):
                ssl = slice(s * 512, (s + 1) * 512)
                nc.tensor.matmul(
                    inv_ps[0:64, ssl], lhsT=ones2[0:64, :], rhs=psq[0:64, ssl],
                    start=True, stop=True,
                )
                nc.tensor.matmul(
                    inv_ps[64:128, ssl], lhsT=ones2[64:128, :], rhs=psq[64:128, ssl],
                    start=True, stop=True, tile_position=(64, 64),
                )
            inv_pss.append(inv_ps)
        if USE_ARS:
            for i, c in enumerate(cs):  # invn = 1/sqrt(nrm2) single pass
                i2 = nc.scalar.activation(
                    out=inv_pss[i], in_=inv_pss[i], func=AF.Abs_reciprocal_sqrt
                )
                m_exps.append(i2)
        else:
            m_lns = []
            for i, c in enumerate(cs):  # batched Ln (in-place on PSUM)
                i1 = nc.scalar.activation(out=inv_pss[i], in_=inv_pss[i], func=AF.Ln)
                m_lns.append(i1)
            for i, c in enumerate(cs):  # batched Exp -> nrm2^-0.5 (in-place)
                i2 = nc.scalar.activation(
                    out=inv_pss[i], in_=inv_pss[i], func=AF.Exp, scale=-0.5
                )
                add_dep_helper(i2.ins, m_lns[-1].ins, sync=False, reason="act batch")
                m_exps.append(i2)
        for i, c in enumerate(cs):
            sl = slice(c * 2048, (c + 1) * 2048)
            nc.vector.tensor_tensor(
                out=mT[:, sl], in0=mp[:, sl], in1=inv_pss[i], op=OP.mult
            )

    # ---- w_tm1 in one big DMA ----
    wp_sb = big.tile([128, N], F32, name="wp_sb")
    nc.sync.dma_start(out=wp_sb, in_=wprev)

    # ---- content scores + exp (u = exp(beta * cos)), row sums via accum ----
    u_sb = big.tile([128, N], F32, name="u_sb")
    racc = small.tile([128, NCH], F32, name="racc")
    u_exps = []
    for j in range(NCH):
        ps_n = psum.tile([128, CH], F32, name="ps_n", tag="ps")
        if j < 4:
            kTv, rows, base = kT[0:64, :], slice(0, 64), j * CH
        else:
            kTv, rows, base = kT[64:128, :], slice(64, 128), (j - 4) * CH
        for b in range(4):
            nc.tensor.matmul(
                ps_n[:, b * 512 : (b + 1) * 512],
                lhsT=kTv,
                rhs=mT[rows, base + b * 512 : base + (b + 1) * 512],
                start=True,
                stop=True,
            )
        ue = nc.scalar.activation(
            out=u_sb[:, j * CH : (j + 1) * CH],
            in_=ps_n,
            func=AF.Exp,
            scale=bscale,
            accum_out=racc[:, j : j + 1],
        )
        u_exps.append(ue)
    add_dep_helper(u_exps[0].ins, m_exps[-1].ins, sync=False, reason="act batch")

    # ---- R = sum(u); gs = g / ((1-g_clamped) * R) ----
    R = small.tile([128, 1], F32, name="R")
    nc.vector.tensor_reduce(out=R, in_=racc, axis=AX.X, op=OP.add)
    gcl = small.tile([128, 1], F32, name="gcl")
    nc.vector.tensor_scalar(out=gcl, in0=g_sb, scalar1=GMAX, scalar2=None, op0=OP.min)
    om = small.tile([128, 1], F32, name="om")
    nc.vector.tensor_scalar(
        out=om, in0=gcl, scalar1=-1.0, scalar2=1.0, op0=OP.mult, op1=OP.add
    )
    omr = small.tile([128, 1], F32, name="omr")
    nc.vector.tensor_tensor(out=omr, in0=om, in1=R, op=OP.mult)
    iomr = small.tile([128, 1], F32, name="iomr")
    nc.vector.reciprocal(out=iomr, in_=omr)
    gs = small.tile([128, 1], F32, name="gs")
    nc.vector.tensor_tensor(out=gs, in0=g_sb, in1=iomr, op=OP.mult)

    # ---- v_hat = gs*u + w_tm1  -> bf16 (slot shared with mp) ----
    vb = big.tile([128, N], BF16, name="vb", tag="mp_vb")
    stts = []
    for j in range(NCH):
        sl = slice(j * CH, (j + 1) * CH)
        stts.append(nc.vector.scalar_tensor_tensor(
            out=vb[:, sl],
            in0=u_sb[:, sl],
            scalar=gs,
            in1=wp_sb[:, sl],
            op0=OP.mult,
            op1=OP.add,
        ))

    # keep the PE HAM warm across the R-barrier: dummy weight loads tied
    # to the v_hat chain so the scheduler spreads them through the gap
    for j in range(NCH):
        dw = nc.tensor.ldweights(kT[0:64, :])
        add_dep_helper(dw.ins, stts[j].ins, sync=False, reason="ham warm")

    # ---- per-sample diag(s_j) stationaries (bf16) for the shift conv ----
    diag = const.tile([128, 3, 128], BF16, name="diag")
    for t in range(3):
        nc.vector.tensor_scalar(
            out=diag[:, t, :], in0=ident, scalar1=s3[:, t : t + 1], scalar2=None,
            op0=OP.mult,
        )

    # ---- conv (PE bf16, accumulating taps) -> ln (batched) ----
    eps_b = small.tile([128, 1], F32, name="eps_b")
    nc.vector.memset(eps_b, 1e-6)
    l_sb = u_sb  # reuse: u dead after v_hat
    lns = []
    for j in range(NCH):
        ps_c = psum.tile([128, CH], F32, name="ps_c", tag="ps")
        # tap order [1, 0, 2]: tap 1 (no shift) covers every column -> start
        for ti, t in enumerate([1, 0, 2]):
            for b in range(4):
                o0, o1 = b * 512, (b + 1) * 512
                r0 = j * CH + b * 512 + t - 1
                r1 = r0 + 512
                if r0 < 0:
                    o0, r0 = o0 + 1, 0
                if r1 > N:
                    o1, r1 = o1 - 1, N
                nc.tensor.matmul(
                    ps_c[:, o0:o1],
                    lhsT=diag[:, t, :],
                    rhs=vb[:, r0:r1],
                    start=(ti == 0),
                    stop=(ti == 2),
                    skip_group_check=True,
                )
        li = nc.scalar.activation(
            out=l_sb[:, j * CH : (j + 1) * CH],
            in_=ps_c,
            func=AF.Ln,
            scale=om,
            bias=eps_b,
        )
        add_dep_helper(li.ins, u_exps[-1].ins, sync=False, reason="act batch")
        lns.append(li)

    # ---- sharpen: w = exp(gamma * l) (batched Exp), row sums via accum ----
    w_sb = wp_sb  # reuse: w_tm1 dead after v_hat
    sacc = small.tile([128, 4], F32, name="sacc")
    exp2s = []
    for j in range(4):
        sl = slice(j * 2 * CH, (j + 1) * 2 * CH)
        e2 = nc.scalar.activation(
            out=w_sb[:, sl],
            in_=l_sb[:, sl],
            func=AF.Exp,
            scale=gamma,
            accum_out=sacc[:, j : j + 1],
        )
        add_dep_helper(e2.ins, lns[-1].ins, sync=False, reason="act batch")
        exp2s.append(e2)

    # ---- final normalize (split DVE/ACT) + store ----
    Ssum = small.tile([128, 1], F32, name="Ssum")
    nc.vector.tensor_reduce(out=Ssum, in_=sacc, axis=AX.X, op=OP.add)
    sinv = small.tile([128, 1], F32, name="sinv")
    nc.vector.reciprocal(out=sinv, in_=Ssum)
    for j in range(NCH):
        sl = slice(j * CH, (j + 1) * CH)
        nc.vector.tensor_scalar(
            out=w_sb[:, sl], in0=w_sb[:, sl], scalar1=sinv, scalar2=None,
            op0=OP.mult,
        )
        nc.sync.dma_start(out=out[:, sl], in_=w_sb[:, sl])


def build_program():
    nc = bacc.Bacc(
        "TRN2", target_bir_lowering=False, debug=False, num_devices=NCORES
    )
    h = nc.dram_tensor("h", [BL, H], F32, kind="ExternalInput").ap()
    wprev = nc.dram_tensor("wprev", [BL, N], F32, kind="ExternalInput").ap()
    mtp = nc.dram_tensor("mtp", [128, HALF], F32, kind="ExternalInput").ap()
    wcat = nc.dram_tensor("wcat", [H, 70], F32, kind="ExternalInput").ap()
    bcat = nc.dram_tensor("bcat", [1, 70], F32, kind="ExternalInput").ap()
    out = nc.dram_tensor("out", [BL, N], F32, kind="ExternalOutput").ap()
    with tile.TileContext(nc) as tc, ExitStack() as ctx:
        _body(ctx, tc, out, h, wprev, mtp, wcat, bcat)
    nc.compile()
    return nc


_CACHED_NC = None


def _pack_host_inputs(
    h_t, w_tm1, m_t, Wk, bk, Wb, bb, Wg, bg, Ws, bs, Wm, bm
):
    wcat = np.concatenate([Wk, Wb, Wg, Ws, Wm], axis=0).astype(np.float32)  # [70, H]
    bcat = np.concatenate([bk, bb, bg, bs, bm], axis=0).astype(np.float32)  # [70]
    wcat_t = np.ascontiguousarray(wcat.T)  # [H, 70]
    bcat2 = np.ascontiguousarray(bcat.reshape(1, 70))
    mt = np.asarray(m_t, dtype=np.float32).T  # [64, N] view
    mtp = np.ascontiguousarray(
        np.concatenate([mt[:, :HALF], mt[:, HALF:]], axis=0)
    )  # [128, 8192]
    in_maps = []
    for c in range(NCORES):
        sl = slice(c * BL, (c + 1) * BL)
        in_maps.append(
            {
                "h": np.ascontiguousarray(np.asarray(h_t[sl], dtype=np.float32)),
                "wprev": np.ascontiguousarray(np.asarray(w_tm1[sl], dtype=np.float32)),
                "mtp": mtp,
                "wcat": wcat_t,
                "bcat": bcat2,
            }
        )
    return in_maps


def kernel(**inputs) -> np.ndarray:
    global _CACHED_NC
    if _CACHED_NC is None:
        _CACHED_NC = build_program()
    in_maps = _pack_host_inputs(**inputs)
    res = run_bass_kernel_spmd(_CACHED_NC, in_maps, core_ids=list(range(NCORES)))
    return np.concatenate([res.results[c]["out"] for c in range(NCORES)], axis=0)
